# revision 1
# baseline (speedup 1.0000x reference)
"""Trainium2 Bass kernel for nn_BilinearEquivariantLayer.

Pipeline (per core c of 8, SPMD):
  stage 1: A_pos[k] = P[k] @ V[k] for k in {2c, 2c+1}      (k-sharded)
  AllToAll: redistribute A_pos so core c holds all k for its 64-col r-block
  stage 2: irfft over k as per-(dchunk,rc) matmuls vs CIR -> A_real in SBUF
  stage 3: W1A/W2A = W @ A_real  (own r-block; bf16 out)
  AllGather: W1A across cores -> full R (overlapped with W2A pass)
  stage 4: bilinear U[t,h] = W2A[t,h].T @ W1A[t,h]  (own s-block, full r)
  stage 5: fused rfft+mixer as one (248->256) matmul (G), direct to output
"""
import sys
sys.path.insert(0, "/opt/trn_rl_repo")
import os
import numpy as np
from concourse import bass, bacc, tile, mybir
from concourse import bass_utils

NCORES = 8
K, D, N, R, H, dproj = 16, 512, 1024, 512, 8, 128
T = 2 * K - 1           # 31
KL = K // NCORES        # 2 k's per core
RC = R // NCORES        # 64 r-cols per core
F32 = mybir.dt.float32
F32R = mybir.dt.float32r
BF16 = mybir.dt.bfloat16

_CACHE = {}


def _build():
    nc = bacc.Bacc("TRN2", target_bir_lowering=False, debug=False,
                   num_devices=NCORES)
    pt = nc.dram_tensor("pt", [KL, 2, N, D], F32R, kind="ExternalInput").ap()
    v = nc.dram_tensor("v", [KL, N, R], F32R, kind="ExternalInput").ap()
    w1t = nc.dram_tensor("w1t", [D, H * dproj], F32R, kind="ExternalInput").ap()
    w2t = nc.dram_tensor("w2t", [D, H * dproj], F32R, kind="ExternalInput").ap()
    cir = nc.dram_tensor("cir", [2 * K, 32], F32R,
                         kind="ExternalInput").ap()
    g = nc.dram_tensor("g", [2, 124, 256], BF16, kind="ExternalInput").ap()
    o = nc.dram_tensor("o", [2, K, H, RC, R], F32, kind="ExternalOutput").ap()

    # round-robin DMA trigger issue across the DMA-capable engines
    dma_cnt = [0]

    def dma(out, in_):
        eng = (nc.sync, nc.scalar, nc.gpsimd)[dma_cnt[0] % 3]
        dma_cnt[0] += 1
        eng.dma_start(out=out, in_=in_)

    with tile.TileContext(nc) as tc:
        with tc.tile_pool(name="dram", bufs=1, space="DRAM") as dram:
            a2a_ins = [dram.tile([NCORES, D, RC], F32R,
                                  name=f"a2ain{i}") for i in range(4)]
            a2a_outs = [dram.tile([NCORES, D, RC], F32R,
                                   name=f"a2aout{i}") for i in range(4)]
            w1a_loc = dram.tile([H, dproj, 32, RC], BF16)
            w1a_ag = dram.tile([NCORES, H, dproj, 32, RC], BF16,
                               addr_space="Shared")
            u_dram = dram.tile([2, 124, RC, R], BF16)

            # ---- stage 1: A_pos = P @ V (own k's) -> a2a_in
            sc1 = nc.named_scope("st1"); sc1.__enter__()
            with tc.tile_pool(name="s1", bufs=1) as s1, \
                 tc.tile_pool(name="s1c", bufs=3) as s1c, \
                 tc.tile_pool(name="ps1", bufs=3, space="PSUM") as ps1p:
                pt_sb = s1.tile([128, KL, 2, 8, D], F32R)
                v_sb = s1.tile([128, KL, 8, R], F32R)
                for kl in range(KL):
                    for nc0 in range(0, 8, 2):
                        dma(v_sb[:, kl, nc0:nc0 + 2, :],
                            v[kl, nc0 * 128:(nc0 + 2) * 128, :].rearrange(
                                "(a p) d -> p a d", p=128))
                        for ri in range(2):
                            dma(pt_sb[:, kl, ri, nc0:nc0 + 2, :],
                                pt[kl, ri, nc0 * 128:(nc0 + 2) * 128,
                                   :].rearrange("(a p) d -> p a d", p=128))
                for kl in range(KL):
                    for ri in range(2):
                        for dc in range(4):
                            ps1 = ps1p.tile([128, R], F32, tag="ps1")
                            for nci in range(8):
                                nc.tensor.matmul(
                                    ps1[:],
                                    pt_sb[:, kl, ri, nci,
                                          dc * 128:(dc + 1) * 128],
                                    v_sb[:, kl, nci, :],
                                    start=(nci == 0), stop=(nci == 7))
                            cp1 = s1c.tile([128, R], F32R, tag="cp1")
                            nc.scalar.copy(cp1[:], ps1[:])
                            dma(a2a_ins[kl * 2 + ri][
                                    :, dc * 128:(dc + 1) * 128,
                                    :].transpose([1, 0, 2]),
                                cp1[:].rearrange("p (rb rc) -> p rb rc", rb=8))
                        nc.gpsimd.collective_compute(
                            "AllToAll", mybir.AluOpType.bypass,
                            replica_groups=[list(range(NCORES))],
                            ins=[a2a_ins[kl * 2 + ri].opt()],
                            outs=[a2a_outs[kl * 2 + ri].opt()])

            sc1.__exit__(None, None, None)
            # pools spanning stages 2+3 (w2a_sb lives to stage 4)
            w2actx = tc.tile_pool(name="w2ap", bufs=1)
            w2ap = w2actx.__enter__()
            w2a_sb = w2ap.tile([dproj, H, 32, RC], BF16)
            with tc.tile_pool(name="mid", bufs=1) as mid:
                # ar_sb[dl, dc, rcb, t, rc16] = A_real, written by stage 2
                ar_sb = mid.tile([128, 4, 32, RC], F32R)
                w1t_sb = mid.tile([128, 4, H * dproj], F32R)
                w2t_sb = mid.tile([128, 4, H * dproj], F32R)
                cir_sb = mid.tile([2 * K, 32], F32R)
                dma(cir_sb[:], cir[:, :])
                for dc in range(4):
                    dma(w1t_sb[:, dc, :], w1t[dc * 128:(dc + 1) * 128, :])
                    dma(w2t_sb[:, dc, :], w2t[dc * 128:(dc + 1) * 128, :])

                # ---- stage 2: irfft; A_pos slices stationary, CIR moving
                sc2 = nc.named_scope("st2"); sc2.__enter__()
                s2ctx = tc.tile_pool(name="s2", bufs=2)
                s2 = s2ctx.__enter__()
                ps2ctx = tc.tile_pool(name="ps2", bufs=6, space="PSUM")
                ps2p = ps2ctx.__enter__()
                for dc in range(4):
                    a2a_sb = s2.tile([2 * K, 128, RC], F32R, tag="a2a_sb")
                    for klri in range(4):
                        dma(a2a_sb[klri * 8:(klri + 1) * 8].rearrange(
                                "k b c -> k (b c)"),
                            a2a_outs[klri][:].rearrange(
                                "s dd rc -> s (dd rc)")[
                                :, dc * 8192:(dc + 1) * 8192])
                    for rc in range(RC):
                        ps2 = ps2p.tile([128, 32], F32, tag="ps2")
                        nc.tensor.matmul(
                            ps2[:], a2a_sb[:, :, rc], cir_sb[:],
                            start=True, stop=True)
                        nc.vector.tensor_copy(
                            ar_sb[:, dc, :, rc], ps2[:])
                ps2ctx.__exit__(None, None, None)
                s2ctx.__exit__(None, None, None)
                sc2.__exit__(None, None, None)

                # ---- stage 3: W1A -> bf16 DRAM + AG; W2A -> SBUF resident
                sc3 = nc.named_scope("st3"); sc3.__enter__()
                with tc.tile_pool(name="s3c", bufs=4) as s3c, \
                     tc.tile_pool(name="ps3", bufs=4, space="PSUM") as ps3p:
                    for h in range(H):
                        for tb in range(4):
                            ps3 = ps3p.tile([128, 512], F32, tag="ps3")
                            for dc in range(4):
                                nc.tensor.matmul(
                                    ps3[:],
                                    w1t_sb[:, dc, h * 128:(h + 1) * 128],
                                    ar_sb[:, dc,
                                          tb * 8:(tb + 1) * 8].rearrange(
                                        "p t rc -> p (t rc)"),
                                    start=(dc == 0), stop=(dc == 3))
                            wcast = s3c.tile([128, 512], BF16, tag="wcast")
                            nc.scalar.copy(wcast[:], ps3[:])
                            dma(w1a_loc[h, :, tb * 8:(tb + 1) * 8, :],
                                wcast[:].rearrange("p (t rc) -> p t rc",
                                                   t=8))
                    nc.gpsimd.collective_compute(
                        "AllGather", mybir.AluOpType.bypass,
                        replica_groups=[list(range(NCORES))],
                        ins=[w1a_loc.opt()], outs=[w1a_ag.opt()])
                    for h in range(H):
                        for tb in range(4):
                            ps3 = ps3p.tile([128, 512], F32, tag="ps3")
                            for dc in range(4):
                                nc.tensor.matmul(
                                    ps3[:],
                                    w2t_sb[:, dc, h * 128:(h + 1) * 128],
                                    ar_sb[:, dc,
                                          tb * 8:(tb + 1) * 8].rearrange(
                                        "p t rc -> p (t rc)"),
                                    start=(dc == 0), stop=(dc == 3))
                            nc.vector.tensor_copy(
                                w2a_sb[:, h, tb * 8:(tb + 1) * 8, :].rearrange(
                                    "p t rc -> p (t rc)"),
                                ps3[:])

            sc3.__exit__(None, None, None)
            # ---- stage 4: bilinear -> u_dram
            sc4 = nc.named_scope("st4"); sc4.__enter__()
            with tc.tile_pool(name="s4", bufs=1) as s4, \
                 tc.tile_pool(name="s4w", bufs=2) as s4w, \
                 tc.tile_pool(name="s4c", bufs=4) as s4c, \
                 tc.tile_pool(name="ps4", bufs=4, space="PSUM") as ps4p:
                for h in range(H):
                    w1a_sb = s4w.tile([dproj, 32, R], BF16, tag="w1a_sb")
                    for rb in range(NCORES):
                        dma(w1a_sb[:, :, rb * RC:(rb + 1) * RC],
                            w1a_ag[rb, h])
                    for t0 in range(0, T, 4):
                        tw = min(4, T - t0)
                        cp4 = s4c.tile([64, 4 * R], BF16, tag="cp4")
                        for tt in range(tw):
                            ps4 = ps4p.tile([64, R], F32, tag="ps4")
                            nc.tensor.matmul(
                                ps4[:], w2a_sb[:, h, t0 + tt, :],
                                w1a_sb[:, t0 + tt, :],
                                start=True, stop=True)
                            nc.vector.tensor_copy(
                                cp4[:, tt * R:(tt + 1) * R], ps4[:])
                        p0 = (h % 4) * T + t0
                        dma(u_dram[h // 4, p0:p0 + tw].transpose([1, 0, 2]),
                            cp4[:, 0:tw * R].rearrange(
                                "p (a r) -> p a r", a=tw))

            w2actx.__exit__(None, None, None)
            sc4.__exit__(None, None, None)
            # ---- stage 5: out = G.T @ U
            sc5 = nc.named_scope("st5"); sc5.__enter__()
            u_flat = u_dram[:].rearrange("c p sc r -> c p (sc r)")
            with tc.tile_pool(name="s5", bufs=1) as s5, \
                 tc.tile_pool(name="s5r", bufs=4) as s5r, \
                 tc.tile_pool(name="ps5", bufs=4, space="PSUM") as ps5p:
                g_sb = s5.tile([124, 2, 256], BF16)
                for cc in range(2):
                    dma(g_sb[:, cc, :], g[cc, :, :])
                o_flat = o.rearrange("m k j sc r -> m (k j) (sc r)")
                for fc0 in range(0, 64, 2):
                    urhs = s5r.tile([124, 2, 1024], BF16, tag="urhs")
                    for cc in range(2):
                        dma(urhs[:, cc, :],
                            u_flat[cc, :, fc0 * 512:(fc0 + 2) * 512])
                    for mc in range(2):
                        cp5 = s5r.tile([128, 1024], F32, tag="cp5")
                        for f in range(2):
                            ps5 = ps5p.tile([128, 512], F32, tag="ps5")
                            for cc in range(2):
                                nc.tensor.matmul(
                                    ps5[:],
                                    g_sb[:, cc, mc * 128:(mc + 1) * 128],
                                    urhs[:, cc, f * 512:(f + 1) * 512],
                                    start=(cc == 0), stop=(cc == 1))
                            nc.vector.tensor_copy(
                                cp5[:, f * 512:(f + 1) * 512], ps5[:])
                        dma(o_flat[mc, :, fc0 * 512:(fc0 + 2) * 512], cp5[:])

            sc5.__exit__(None, None, None)
    nc.compile()
    return nc


def _host_prep(P_real, P_imag, V, W1, W2, mixer_real, mixer_imag):
    P_real = np.asarray(P_real, np.float32)
    P_imag = np.asarray(P_imag, np.float32)
    V = np.asarray(V, np.float32)
    W1 = np.asarray(W1, np.float32)
    W2 = np.asarray(W2, np.float32)
    mr = np.asarray(mixer_real, np.float32)
    mi = np.asarray(mixer_imag, np.float32)

    pt_all = np.stack([P_real.transpose(0, 2, 1),
                       P_imag.transpose(0, 2, 1)], axis=1)  # (K, 2, N, D)
    w1t = np.ascontiguousarray(W1.reshape(H * dproj, D).T)
    w2t = np.ascontiguousarray(W2.reshape(H * dproj, D).T)

    t_idx, k_idx = np.arange(T), np.arange(K)
    ang = 2 * np.pi * np.outer(k_idx, t_idx) / T
    scale = np.where(k_idx[:, None] == 0, 1.0, 2.0) / T
    # contraction row order p = kl*16 + ri*8 + src, global k = 2*src + kl
    cir = np.zeros((2 * K, 32), np.float32)
    cr_k = np.cos(ang) * scale
    ci_k = -np.sin(ang) * scale
    for p in range(2 * K):
        kl_, ri_, src_ = p // 16, (p // 8) % 2, p % 8
        k_ = 2 * src_ + kl_
        cir[p, :T] = cr_k[k_] if ri_ == 0 else ci_k[k_]

    cos2, sin2 = np.cos(ang), np.sin(ang)  # (K, T)
    G = np.empty((H, T, 2, K, H), np.float32)
    G[:, :, 0] = (np.einsum('kt,ij->itkj', cos2, mr)
                  + np.einsum('kt,ij->itkj', sin2, mi))
    G[:, :, 1] = (np.einsum('kt,ij->itkj', cos2, mi)
                  - np.einsum('kt,ij->itkj', sin2, mr))
    from ml_dtypes import bfloat16 as _bf16
    g = np.ascontiguousarray(G.reshape(2, 124, 256)).astype(_bf16)

    in_maps = []
    for c in range(NCORES):
        in_maps.append({
            "pt": np.ascontiguousarray(pt_all[2 * c:2 * c + 2]),
            "v": np.ascontiguousarray(V[2 * c:2 * c + 2]),
            "w1t": w1t, "w2t": w2t, "cir": cir, "g": g,
        })
    return in_maps


def _assemble(outs):
    res = np.empty((K, R, R, H), np.complex64)
    for c in range(NCORES):
        oc = outs[c]  # (2, K, H, RC, R)
        res[:, :, c * RC:(c + 1) * RC, :] = (
            oc[0] + 1j * oc[1]).transpose(0, 3, 2, 1)
    return res


def _enable_axon_trace():
    """Dev-only: register the NTFF profile hook (missing antenv.axon_hooks)
    and stub the artifact upload so run_bass_kernel_spmd(trace=True) works."""
    import types
    if "antenv.axon_hooks" not in sys.modules:
        m = types.ModuleType("antenv.axon_hooks")
        m._hook = None
        m.set_axon_ntff_profile_hook = lambda h: setattr(m, "_hook", h)
        m.get_axon_ntff_profile_hook = lambda: m._hook
        sys.modules["antenv.axon_hooks"] = m
        import antenv
        antenv.axon_hooks = m
        from trn_agent_boot.trn_boot import _ntff_profile_via_ctypes
        hook = _ntff_profile_via_ctypes("/opt/axon/libaxon_pjrt.so")
        m._hook = hook
    bass_utils.upload_artifacts = lambda tmpdir: f"local:{tmpdir}"


def kernel(P_real, P_imag, V, W1, W2, mixer_real, mixer_imag):
    if "nc" not in _CACHE:
        _CACHE["nc"] = _build()
    nc = _CACHE["nc"]
    in_maps = _host_prep(P_real, P_imag, V, W1, W2, mixer_real, mixer_imag)

    if os.environ.get("KSIM"):
        from concourse.bass_interp import MultiCoreSim
        sim = MultiCoreSim(nc, num_cores=NCORES, num_workers=NCORES)
        for c in range(NCORES):
            for k_, arr in in_maps[c].items():
                sim.cores[c].tensor(k_)[:] = arr
        sim.simulate(check_with_hw=False)
        outs = [np.array(sim.cores[c].tensor("o")) for c in range(NCORES)]
        return _assemble(outs)

    trace = bool(os.environ.get("KTRACE"))
    if trace:
        _enable_axon_trace()
    res = bass_utils.run_bass_kernel_spmd(
        nc, in_maps, core_ids=list(range(NCORES)), trace=trace,
        tmpdir=os.environ.get("KTRACE_DIR") or None)
    if trace:
        print(f"HW exec time: {res.exec_time_ns} ns")
        _CACHE["exec_time_ns"] = res.exec_time_ns
        _CACHE["results"] = res
    outs = [res.results[c]["o"] for c in range(NCORES)]
    return _assemble(outs)

